# revision 34
# baseline (speedup 1.0000x reference)
"""Trainium2 Bass kernel for nn_BaseEncoder (ragged entity-pair encoder).

Contract: kernel(**inputs) takes the FULL unsharded inputs (numpy) and
returns the FULL output [B, Q, E, E, R] float32.

Sharding: B*Q = 8 independent (batch, query) pairs -> one per NeuronCore.

Host-side prep per core (numpy; gather/layout + the O(E^2*NH*L) pair
normalizer the baseline already computed host-side for S):
  - gather the E*M mention rows of the per-query attention, sum over the
    M=2 mentions, and form the normalized pair weights
      muln[l, (e,f)] = sum_h at[l,h,e] at[l,h,f] / S[e,f]
    (the /M, /NH scalings cancel in the row normalization). Only the 768
    unique cols are sent: chunk0 = (e<16, all f), chunk1 = (e>=16,f>=16);
    the rest follows from (e,f) symmetry.
  - entity-bias rows ep_w = (mean-mention ent) @ W_w[:H]  [E, H]
  - W_head/W_tail ctx halves (rows H:2H) and prototypes, pre-tiled.

Device kernel per core — PE pipeline with double-fp8 matmuls for the two
big contractions (safe: the ctx/Z signal is ~20x smaller than the ep
entity bias, so fp8 error is diluted well below the gate; verified
numerically at ~2.4e-3 final rel err). Scale bookkeeping: seq*1,
muln*1024 -> ctx' = 1024*ctx; cn8 = ctx'/64 = 16*ctx; W*64 -> Z' =
1024*Z; ep sent *1024; tanh applies scale 1/1024.
  ctx'[h', ef] = sum_l seq8[l, h'] muln8[l, ef]       (TensorE fp8 x2)
  cn8          = ctx' / 64                            (ScalarE copy)
  Z'[h'', ef]  = sum_h' W8[h', h''] cn8[h', ef]       (TensorE fp8 x2)
  pre[h'', ef] = Z'[sym(e,f)] + ep'[e or f]           (VectorE, fp16)
  cand         = tanh(pre / 1024)                     (ScalarE)
  sc[rp, ef]   = sum_d protoT[d, rp] cand[d, ef]      (TensorE fp16)
  out          = sc                                   (VectorE copy + DMA)
Host: max over the P support prototypes + reshape.
"""

import numpy as np

B, Q, L, H, E, M, R, P, NH = 2, 4, 1024, 768, 32, 2, 5, 10, 12
NCORES = 8
LT = L // 128          # 8 l-tiles
HT = H // 128          # 6 tiles of 128 along a hidden dim
EF = E * E             # 1024 entity pairs
RP = R * P             # 50 prototype rows
EC = E // 2            # 16 e-rows per chunk
C0 = EC * E            # 512 unique cols in chunk 0 (e<16, all f)
C1 = EC * EC           # 256 unique cols in chunk 1 (e>=16, f>=16)
NG = 2 * HT            # 12 projection groups (w, ht2)

_CACHE = {}


def _build_program():
    import concourse.mybir as mybir
    import concourse.tile as tile
    from concourse import bacc

    f8 = mybir.dt.float8e4
    f16 = mybir.dt.float16
    f32 = mybir.dt.float32
    nc = bacc.Bacc("TRN2", target_bir_lowering=False, debug=False,
                   num_devices=NCORES)

    sm0_d = nc.dram_tensor("sm0", [128, LT, H + C0], f8,
                           kind="ExternalInput").ap()
    mu1_d = nc.dram_tensor("mu1", [128, LT, C1], f8,
                           kind="ExternalInput").ap()
    whc_d = nc.dram_tensor("whc", [128, HT // 2, 2, H], f8,
                           kind="ExternalInput").ap()
    wtc_d = nc.dram_tensor("wtc", [128, HT // 2, 2, H], f8,
                           kind="ExternalInput").ap()
    ptT_d = nc.dram_tensor("ptT", [128, NG, RP], f16,
                           kind="ExternalInput").ap()
    ep_d = nc.dram_tensor("ep", [128, NG, E], f16,
                          kind="ExternalInput").ap()
    out_d = nc.dram_tensor("out", [RP, EF], f16, kind="ExternalOutput").ap()

    with tile.TileContext(nc) as tc:
        _emit(tc, mybir, sm0_d, mu1_d, whc_d, wtc_d, ptT_d, ep_d, out_d)

    nc.compile()
    return nc


def _emit(tc, mybir, sm0_d, mu1_d, whc_d, wtc_d, ptT_d, ep_d, out_d):
    nc = tc.nc
    f8 = mybir.dt.float8e4
    f16 = mybir.dt.float16
    f32 = mybir.dt.float32
    Act = mybir.ActivationFunctionType
    DR = mybir.MatmulPerfMode.DoubleRow

    import contextlib
    ctx = contextlib.ExitStack()
    with ctx:
        big = ctx.enter_context(tc.tile_pool(name="big", bufs=1))
        prep = ctx.enter_context(tc.tile_pool(name="prep", bufs=6))
        psum = ctx.enter_context(tc.tile_pool(name="psum", bufs=1,
                                              space="PSUM"))

        # ---------------- input loads ----------------
        # ONE queue, priority order: a single large DMA already spans all
        # 16 SDMA engines, so extra queues only contend. The seq|mu0
        # stream (interleaved per-lt, per-partition-contiguous) pipelines
        # in 3 chunks; everything else follows in first-use order.
        sm0_sb = big.tile([128, LT, H + C0], f8, tag="sm0_sb")
        mu1_sb = big.tile([128, LT, C1], f8, tag="mu1_sb")
        for a, b in ((0, 4), (4, 8)):
            nc.sync.dma_start(out=sm0_sb[:, a:b, :], in_=sm0_d[:, a:b, :])
        nc.sync.dma_start(out=mu1_sb, in_=mu1_d)
        whc_sb = big.tile([128, HT // 2, 2, H], f8, tag="whc_sb")
        nc.sync.dma_start(out=whc_sb, in_=whc_d)
        ep_sb = big.tile([128, NG, E], f16, tag="ep_sb")
        nc.sync.dma_start(out=ep_sb, in_=ep_d)
        wtc_sb = big.tile([128, HT // 2, 2, H], f8, tag="wtc_sb")
        nc.sync.dma_start(out=wtc_sb, in_=wtc_d)
        ptT_sb = big.tile([128, NG, RP], f16, tag="ptT_sb")
        nc.sync.dma_start(out=ptT_sb, in_=ptT_d)

        # ---------------- SBUF result tiles ----------------
        cn0 = big.tile([128, HT // 2, 2, C0], f8, tag="cn0")
        cn1 = big.tile([128, HT // 2, 2, C1], f8, tag="cn1")
        # zs0 holds only the swapped-read quadrant Z0[e0, f0>=16] (f x e'')
        zs0 = big.tile([128, NG, C1], f16, tag="zs0")
        cand0 = big.tile([128, NG, C0], f16, tag="cand0")
        cand1 = big.tile([128, NG, C0], f16, tag="cand1")
        ob = big.tile([RP, EF], f16, tag="ob")

        # ---------------- ctx: chunk 0 in two 3-bank passes ----------
        # only 3 "ctx" banks so the Z ping-pong gets 5 banks (deep enough
        # that PE never stalls on a Z bank still being read by DVE)
        ctx0_ps = [psum.tile([128, C0], f32, tag="ctx", bufs=3,
                             name=f"ctx0_{ht}") for ht in range(HT)]
        # pass A is lt-major (first DMA chunk covers lt0-1 for all ht);
        # pass B is ht-major so each cn0 copy fires as its ht completes
        for pr in range(LT // 2):
            for ht in range(3):
                nc.tensor.matmul(
                    ctx0_ps[ht],
                    sm0_sb[:, 2 * pr:2 * pr + 2, ht * 128:(ht + 1) * 128],
                    sm0_sb[:, 2 * pr:2 * pr + 2, H:],
                    start=(pr == 0), stop=(pr == LT // 2 - 1),
                    perf_mode=DR)
        for ht in range(3):
            nc.scalar.mul(cn0[:, ht // 2, ht % 2, :], ctx0_ps[ht],
                          1.0 / 64.0)
        for ht in range(3, HT):
            for pr in range(LT // 2):
                nc.tensor.matmul(
                    ctx0_ps[ht],
                    sm0_sb[:, 2 * pr:2 * pr + 2, ht * 128:(ht + 1) * 128],
                    sm0_sb[:, 2 * pr:2 * pr + 2, H:],
                    start=(pr == 0), stop=(pr == LT // 2 - 1),
                    perf_mode=DR)
            nc.scalar.mul(cn0[:, ht // 2, ht % 2, :], ctx0_ps[ht],
                          1.0 / 64.0)

        # ---------------- chunk-1 ctx interleaved with Z0 ----------------
        ctx1_ps = [psum.tile([128, C1], f32, tag="ctx", bufs=3,
                             name=f"ctx1_{ht}") for ht in range(HT)]

        def emit_ctx1(i):
            ht, pr = divmod(i, LT // 2)
            nc.tensor.matmul(
                ctx1_ps[ht],
                sm0_sb[:, 2 * pr:2 * pr + 2, ht * 128:(ht + 1) * 128],
                mu1_sb[:, 2 * pr:2 * pr + 2, :],
                start=(pr == 0), stop=(pr == LT // 2 - 1), perf_mode=DR)
            if pr == LT // 2 - 1:
                nc.scalar.mul(cn1[:, ht // 2, ht % 2, :], ctx1_ps[ht],
                              1.0 / 64.0)

        def emit_z(g, cn, width):
            w, ht2 = divmod(g, HT)
            wsb = whc_sb if w == 0 else wtc_sb
            ps = psum.tile([128, width], f32, tag="z", bufs=5,
                           name=f"z{width}_{g}")
            for pair in range(HT // 2):
                nc.tensor.matmul(
                    ps, wsb[:, pair, :, ht2 * 128:(ht2 + 1) * 128],
                    cn[:, pair, :, :],
                    start=(pair == 0), stop=(pair == HT // 2 - 1),
                    perf_mode=DR)
            return ps

        def emit_pre0(g, zps):
            """pre0[e,f] = Z0[e,f] + ep[e or f] (e<16), then tanh.
            The DVE add reads the Z PSUM directly; the swapped-quadrant
            save (zs0, for chunk-1 reads) also reads it on the DVE."""
            w = g // HT
            z3 = zps.rearrange("p (e f) -> p e f", e=EC)
            nc.vector.tensor_copy(
                zs0[:, g, :].rearrange("p (e f) -> p e f", e=EC),
                z3[:, :, EC:])
            pre = prep.tile([128, EC, E], f16, tag="pre", name=f"pre0_{g}")
            epv = ep_sb[:, g, :]
            if w == 0:
                bias = epv[:, 0:EC, None].broadcast_to([128, EC, E])
            else:
                bias = epv[:, None, :].broadcast_to([128, EC, E])
            nc.vector.tensor_add(pre, z3, bias)
            nc.scalar.activation(
                cand0[:, g, :].rearrange("p (e f) -> p e f", e=EC), pre,
                Act.Tanh, scale=1.0 / 1024.0)

        def emit_pre1(g, zps):
            """pre1[e,f] = Z[sym(e,f)] + ep[e or f] (e>=16), then tanh."""
            w = g // HT
            pre = prep.tile([128, EC, E], f16, tag="pre", name=f"pre1_{g}")
            z1 = zps.rearrange("p (e f) -> p e f", e=EC)
            # swapped read: Z[sym(e,f)] = Z0[f, e] for f<16, from the saved
            # quadrant zs0[f, e-16] laid out (f, e'')
            z0sw = zs0[:, g, :].rearrange("p (f e) -> p e f", f=EC)
            epv = ep_sb[:, g, :]
            if w == 0:
                bias_lo = epv[:, EC:, None].broadcast_to([128, EC, EC])
                bias_hi = bias_lo
            else:
                bias_lo = epv[:, None, 0:EC].broadcast_to([128, EC, EC])
                bias_hi = epv[:, None, EC:].broadcast_to([128, EC, EC])
            nc.gpsimd.tensor_add(pre[:, :, 0:EC], z0sw, bias_lo)
            nc.vector.tensor_add(pre[:, :, EC:], z1, bias_hi)
            nc.scalar.activation(
                cand1[:, g, :].rearrange("p (e f) -> p e f", e=EC), pre,
                Act.Tanh, scale=1.0 / 1024.0)

        ci = 0
        for g in range(NG):
            for _ in range(2):
                emit_ctx1(ci)
                ci += 1
            zps = emit_z(g, cn0, C0)
            emit_pre0(g, zps)

        # ---------------- scores-0, then Z1 + scores-1 ----------------
        sc0 = psum.tile([RP, C0], f32, tag="ctx", bufs=3, name="sc0")
        sc1 = psum.tile([RP, C0], f32, tag="ctx", bufs=3, name="sc1")
        for g in range(NG):
            nc.tensor.matmul(sc0, ptT_sb[:, g, :], cand0[:, g, :],
                             start=(g == 0), stop=(g == NG - 1))
        nc.vector.tensor_copy(ob[:, 0:C0], sc0)
        nc.sync.dma_start(out=out_d[:, 0:C0], in_=ob[:, 0:C0])
        for g in range(NG):
            zps = emit_z(g, cn1, C1)
            emit_pre1(g, zps)
            nc.tensor.matmul(sc1, ptT_sb[:, g, :], cand1[:, g, :],
                             start=(g == 0), stop=(g == NG - 1))
        nc.vector.tensor_copy(ob[:, C0:], sc1)
        nc.sync.dma_start(out=out_d[:, C0:], in_=ob[:, C0:])


def _host_prep(sequence_output, attention, W_head, W_tail, prototypes,
               mention_pos):
    """Build the per-core input maps (numpy only)."""
    import ml_dtypes

    f8 = ml_dtypes.float8_e4m3
    seq = np.asarray(sequence_output, dtype=np.float32)
    att = np.asarray(attention, dtype=np.float32)
    wh = np.asarray(W_head, dtype=np.float32)
    wt = np.asarray(W_tail, dtype=np.float32)
    pro = np.asarray(prototypes, dtype=np.float32)
    pos = np.asarray(mention_pos)

    def tile_rows(m, dt=np.float16):  # [T*128, N] -> [128, T, N]
        t = m.shape[0] // 128
        r = m.reshape(t, 128, -1).transpose(1, 0, 2)
        if dt is f8:
            r = np.clip(r, -240.0, 240.0)
        return np.ascontiguousarray(r, dtype=dt)

    def w_tiles(w):  # ctx rows, *64, DoubleRow pairs: [128, 3, 2, H] fp8
        m = np.clip(w[H:] * np.float32(64.0), -240, 240)
        return np.ascontiguousarray(
            m.reshape(HT // 2, 2, 128, H).transpose(2, 0, 1, 3), dtype=f8)

    whc = w_tiles(wh)
    wtc = w_tiles(wt)

    in_maps = []
    for c in range(NCORES):
        b, q = divmod(c, Q)
        p_bq = pos[b, q]                       # [E, M]
        # attention gather + mention-sum: [NH, E, L] (scale dropped)
        g = att[b, q][:, p_bq, :]              # [NH, E, M, L]
        asum = g[:, :, 0, :] + g[:, :, 1, :]   # [NH, E, L]
        # normalized pair weights muln[l, e, f] = 1024 * G / S
        A = np.ascontiguousarray(asum.transpose(2, 1, 0))  # [L, E, NH]
        G = A @ A.transpose(0, 2, 1)                       # [L, E, E]
        S = G.sum(axis=0)                                  # [E, E]
        Gn = G * (np.float32(1024.0) / S)[None]
        # entity means and tanh-bias rows ep_w = 1024 * ent @ W_w[:H]
        ment = seq[b, q][p_bq]                 # [E, M, H]
        ent = (ment[:, 0, :] + ment[:, 1, :]) * np.float32(0.5)
        ep = np.stack([ent @ wh[:H], ent @ wt[:H]]) * np.float32(1024.0)
        # ep layout [128, NG, E]: ep_l[p, w*HT+ht2, e] = ep[w, e, ht2*128+p]
        ep_l = np.ascontiguousarray(
            ep.reshape(2, E, HT, 128).transpose(3, 0, 2, 1).reshape(
                128, NG, E), dtype=np.float16)
        ptT = tile_rows(pro[b].reshape(RP, 2 * H).T)       # [128, NG, RP]
        sm0 = np.concatenate(
            [seq[b, q], Gn[:, :EC, :].reshape(L, C0)], axis=1)  # [L, H+C0]
        in_maps.append({
            "sm0": tile_rows(sm0, f8),
            "mu1": tile_rows(Gn[:, EC:, EC:].reshape(L, C1), f8),
            "whc": whc,
            "wtc": wtc,
            "ptT": ptT,
            "ep": ep_l,
        })
    return in_maps


def kernel(sequence_output, attention, W_head, W_tail, prototypes,
           mention_pos):
    from concourse.bass_utils import run_bass_kernel_spmd

    if "nc" not in _CACHE:
        _CACHE["nc"] = _build_program()
    nc = _CACHE["nc"]

    in_maps = _host_prep(sequence_output, attention, W_head, W_tail,
                         prototypes, mention_pos)
    res = run_bass_kernel_spmd(nc, in_maps, core_ids=list(range(NCORES)))

    out = np.empty((B, Q, E, E, R), dtype=np.float32)
    for c in range(NCORES):
        b, q = divmod(c, Q)
        sc = res.results[c]["out"]             # [RP, EF]
        v = sc.reshape(R, P, 2, EC, E).max(axis=1)   # [R, 2, EC, E]
        out[b, q] = v.reshape(R, E, E).transpose(1, 2, 0)
    return out


# revision 35
# speedup vs baseline: 1.0011x; 1.0011x over previous
"""Trainium2 Bass kernel for nn_BaseEncoder (ragged entity-pair encoder).

Contract: kernel(**inputs) takes the FULL unsharded inputs (numpy) and
returns the FULL output [B, Q, E, E, R] float32.

Sharding: B*Q = 8 independent (batch, query) pairs -> one per NeuronCore.

Host-side prep per core (numpy; gather/layout + the O(E^2*NH*L) pair
normalizer the baseline already computed host-side for S):
  - gather the E*M mention rows of the per-query attention, sum over the
    M=2 mentions, and form the normalized pair weights
      muln[l, (e,f)] = sum_h at[l,h,e] at[l,h,f] / S[e,f]
    (the /M, /NH scalings cancel in the row normalization). Only the 768
    unique cols are sent: chunk0 = (e<16, all f), chunk1 = (e>=16,f>=16);
    the rest follows from (e,f) symmetry.
  - entity-bias rows ep_w = (mean-mention ent) @ W_w[:H]  [E, H]
  - W_head/W_tail ctx halves (rows H:2H) and prototypes, pre-tiled.

Device kernel per core — PE pipeline with double-fp8 matmuls for the two
big contractions (safe: the ctx/Z signal is ~20x smaller than the ep
entity bias, so fp8 error is diluted well below the gate; verified
numerically at ~2.4e-3 final rel err). Scale bookkeeping: seq*1,
muln*1024 -> ctx' = 1024*ctx; cn8 = ctx'/64 = 16*ctx; W*64 -> Z' =
1024*Z; ep sent *1024; tanh applies scale 1/1024.
  ctx'[h', ef] = sum_l seq8[l, h'] muln8[l, ef]       (TensorE fp8 x2)
  cn8          = ctx' / 64                            (ScalarE copy)
  Z'[h'', ef]  = sum_h' W8[h', h''] cn8[h', ef]       (TensorE fp8 x2)
  pre[h'', ef] = Z'[sym(e,f)] + ep'[e or f]           (VectorE, fp16)
  cand         = tanh(pre / 1024)                     (ScalarE)
  sc[rp, ef]   = sum_d protoT[d, rp] cand[d, ef]      (TensorE fp16)
  out          = sc                                   (VectorE copy + DMA)
Host: max over the P support prototypes + reshape.
"""

import numpy as np

B, Q, L, H, E, M, R, P, NH = 2, 4, 1024, 768, 32, 2, 5, 10, 12
NCORES = 8
LT = L // 128          # 8 l-tiles
HT = H // 128          # 6 tiles of 128 along a hidden dim
EF = E * E             # 1024 entity pairs
RP = R * P             # 50 prototype rows
EC = E // 2            # 16 e-rows per chunk
C0 = EC * E            # 512 unique cols in chunk 0 (e<16, all f)
C1 = EC * EC           # 256 unique cols in chunk 1 (e>=16, f>=16)
NG = 2 * HT            # 12 projection groups (w, ht2)

_CACHE = {}


def _build_program():
    import concourse.mybir as mybir
    import concourse.tile as tile
    from concourse import bacc

    f8 = mybir.dt.float8e4
    f16 = mybir.dt.float16
    f32 = mybir.dt.float32
    nc = bacc.Bacc("TRN2", target_bir_lowering=False, debug=False,
                   num_devices=NCORES)

    sm0_d = nc.dram_tensor("sm0", [128, LT, H + C0], f8,
                           kind="ExternalInput").ap()
    mu1_d = nc.dram_tensor("mu1", [128, LT, C1], f8,
                           kind="ExternalInput").ap()
    whc_d = nc.dram_tensor("whc", [128, HT // 2, 2, H], f8,
                           kind="ExternalInput").ap()
    wtc_d = nc.dram_tensor("wtc", [128, HT // 2, 2, H], f8,
                           kind="ExternalInput").ap()
    ptT_d = nc.dram_tensor("ptT", [128, NG, RP], f16,
                           kind="ExternalInput").ap()
    ep_d = nc.dram_tensor("ep", [128, NG, E], f16,
                          kind="ExternalInput").ap()
    out_d = nc.dram_tensor("out", [RP, EF], f32, kind="ExternalOutput").ap()

    with tile.TileContext(nc) as tc:
        _emit(tc, mybir, sm0_d, mu1_d, whc_d, wtc_d, ptT_d, ep_d, out_d)

    nc.compile()
    return nc


def _emit(tc, mybir, sm0_d, mu1_d, whc_d, wtc_d, ptT_d, ep_d, out_d):
    nc = tc.nc
    f8 = mybir.dt.float8e4
    f16 = mybir.dt.float16
    f32 = mybir.dt.float32
    Act = mybir.ActivationFunctionType
    DR = mybir.MatmulPerfMode.DoubleRow

    import contextlib
    ctx = contextlib.ExitStack()
    with ctx:
        big = ctx.enter_context(tc.tile_pool(name="big", bufs=1))
        prep = ctx.enter_context(tc.tile_pool(name="prep", bufs=6))
        psum = ctx.enter_context(tc.tile_pool(name="psum", bufs=1,
                                              space="PSUM"))

        # ---------------- input loads ----------------
        # ONE queue, priority order: a single large DMA already spans all
        # 16 SDMA engines, so extra queues only contend. The seq|mu0
        # stream (interleaved per-lt, per-partition-contiguous) pipelines
        # in 3 chunks; everything else follows in first-use order.
        sm0_sb = big.tile([128, LT, H + C0], f8, tag="sm0_sb")
        mu1_sb = big.tile([128, LT, C1], f8, tag="mu1_sb")
        for a, b in ((0, 4), (4, 8)):
            nc.sync.dma_start(out=sm0_sb[:, a:b, :], in_=sm0_d[:, a:b, :])
        nc.sync.dma_start(out=mu1_sb, in_=mu1_d)
        whc_sb = big.tile([128, HT // 2, 2, H], f8, tag="whc_sb")
        nc.sync.dma_start(out=whc_sb, in_=whc_d)
        ep_sb = big.tile([128, NG, E], f16, tag="ep_sb")
        nc.sync.dma_start(out=ep_sb, in_=ep_d)
        wtc_sb = big.tile([128, HT // 2, 2, H], f8, tag="wtc_sb")
        nc.sync.dma_start(out=wtc_sb, in_=wtc_d)
        ptT_sb = big.tile([128, NG, RP], f16, tag="ptT_sb")
        nc.sync.dma_start(out=ptT_sb, in_=ptT_d)

        # ---------------- SBUF result tiles ----------------
        cn0 = big.tile([128, HT // 2, 2, C0], f8, tag="cn0")
        cn1 = big.tile([128, HT // 2, 2, C1], f8, tag="cn1")
        # zs0 holds only the swapped-read quadrant Z0[e0, f0>=16] (f x e'')
        zs0 = big.tile([128, NG, C1], f16, tag="zs0")
        cand0 = big.tile([128, NG, C0], f16, tag="cand0")
        cand1 = big.tile([128, NG, C0], f16, tag="cand1")
        ob = big.tile([RP, EF], f32, tag="ob")

        # ---------------- ctx: chunk 0 in two 3-bank passes ----------
        # only 3 "ctx" banks so the Z ping-pong gets 5 banks (deep enough
        # that PE never stalls on a Z bank still being read by DVE)
        ctx0_ps = [psum.tile([128, C0], f32, tag="ctx", bufs=3,
                             name=f"ctx0_{ht}") for ht in range(HT)]
        # pass A is lt-major (first DMA chunk covers lt0-1 for all ht);
        # pass B is ht-major so each cn0 copy fires as its ht completes
        for pr in range(LT // 2):
            for ht in range(3):
                nc.tensor.matmul(
                    ctx0_ps[ht],
                    sm0_sb[:, 2 * pr:2 * pr + 2, ht * 128:(ht + 1) * 128],
                    sm0_sb[:, 2 * pr:2 * pr + 2, H:],
                    start=(pr == 0), stop=(pr == LT // 2 - 1),
                    perf_mode=DR)
        for ht in range(3):
            nc.scalar.mul(cn0[:, ht // 2, ht % 2, :], ctx0_ps[ht],
                          1.0 / 64.0)
        for ht in range(3, HT):
            for pr in range(LT // 2):
                nc.tensor.matmul(
                    ctx0_ps[ht],
                    sm0_sb[:, 2 * pr:2 * pr + 2, ht * 128:(ht + 1) * 128],
                    sm0_sb[:, 2 * pr:2 * pr + 2, H:],
                    start=(pr == 0), stop=(pr == LT // 2 - 1),
                    perf_mode=DR)
            nc.scalar.mul(cn0[:, ht // 2, ht % 2, :], ctx0_ps[ht],
                          1.0 / 64.0)

        # ---------------- chunk-1 ctx interleaved with Z0 ----------------
        ctx1_ps = [psum.tile([128, C1], f32, tag="ctx", bufs=3,
                             name=f"ctx1_{ht}") for ht in range(HT)]

        def emit_ctx1(i):
            ht, pr = divmod(i, LT // 2)
            nc.tensor.matmul(
                ctx1_ps[ht],
                sm0_sb[:, 2 * pr:2 * pr + 2, ht * 128:(ht + 1) * 128],
                mu1_sb[:, 2 * pr:2 * pr + 2, :],
                start=(pr == 0), stop=(pr == LT // 2 - 1), perf_mode=DR)
            if pr == LT // 2 - 1:
                nc.scalar.mul(cn1[:, ht // 2, ht % 2, :], ctx1_ps[ht],
                              1.0 / 64.0)

        def emit_z(g, cn, width):
            w, ht2 = divmod(g, HT)
            wsb = whc_sb if w == 0 else wtc_sb
            ps = psum.tile([128, width], f32, tag="z", bufs=5,
                           name=f"z{width}_{g}")
            for pair in range(HT // 2):
                nc.tensor.matmul(
                    ps, wsb[:, pair, :, ht2 * 128:(ht2 + 1) * 128],
                    cn[:, pair, :, :],
                    start=(pair == 0), stop=(pair == HT // 2 - 1),
                    perf_mode=DR)
            return ps

        def emit_pre0(g, zps):
            """pre0[e,f] = Z0[e,f] + ep[e or f] (e<16), then tanh.
            The DVE add reads the Z PSUM directly; the swapped-quadrant
            save (zs0, for chunk-1 reads) also reads it on the DVE."""
            w = g // HT
            z3 = zps.rearrange("p (e f) -> p e f", e=EC)
            nc.vector.tensor_copy(
                zs0[:, g, :].rearrange("p (e f) -> p e f", e=EC),
                z3[:, :, EC:])
            pre = prep.tile([128, EC, E], f16, tag="pre", name=f"pre0_{g}")
            epv = ep_sb[:, g, :]
            if w == 0:
                bias = epv[:, 0:EC, None].broadcast_to([128, EC, E])
            else:
                bias = epv[:, None, :].broadcast_to([128, EC, E])
            nc.vector.tensor_add(pre, z3, bias)
            nc.scalar.activation(
                cand0[:, g, :].rearrange("p (e f) -> p e f", e=EC), pre,
                Act.Tanh, scale=1.0 / 1024.0)

        def emit_pre1(g, zps):
            """pre1[e,f] = Z[sym(e,f)] + ep[e or f] (e>=16), then tanh."""
            w = g // HT
            pre = prep.tile([128, EC, E], f16, tag="pre", name=f"pre1_{g}")
            z1 = zps.rearrange("p (e f) -> p e f", e=EC)
            # swapped read: Z[sym(e,f)] = Z0[f, e] for f<16, from the saved
            # quadrant zs0[f, e-16] laid out (f, e'')
            z0sw = zs0[:, g, :].rearrange("p (f e) -> p e f", f=EC)
            epv = ep_sb[:, g, :]
            if w == 0:
                bias_lo = epv[:, EC:, None].broadcast_to([128, EC, EC])
                bias_hi = bias_lo
            else:
                bias_lo = epv[:, None, 0:EC].broadcast_to([128, EC, EC])
                bias_hi = epv[:, None, EC:].broadcast_to([128, EC, EC])
            nc.gpsimd.tensor_add(pre[:, :, 0:EC], z0sw, bias_lo)
            nc.vector.tensor_add(pre[:, :, EC:], z1, bias_hi)
            nc.scalar.activation(
                cand1[:, g, :].rearrange("p (e f) -> p e f", e=EC), pre,
                Act.Tanh, scale=1.0 / 1024.0)

        ci = 0
        for g in range(NG):
            for _ in range(2):
                emit_ctx1(ci)
                ci += 1
            zps = emit_z(g, cn0, C0)
            emit_pre0(g, zps)

        # ---------------- scores-0, then Z1 + scores-1 ----------------
        sc0 = psum.tile([RP, C0], f32, tag="ctx", bufs=3, name="sc0")
        sc1 = psum.tile([RP, C0], f32, tag="ctx", bufs=3, name="sc1")
        for g in range(NG):
            nc.tensor.matmul(sc0, ptT_sb[:, g, :], cand0[:, g, :],
                             start=(g == 0), stop=(g == NG - 1))
        nc.vector.tensor_copy(ob[:, 0:C0], sc0)
        nc.sync.dma_start(out=out_d[:, 0:C0], in_=ob[:, 0:C0])
        for g in range(NG):
            zps = emit_z(g, cn1, C1)
            emit_pre1(g, zps)
            nc.tensor.matmul(sc1, ptT_sb[:, g, :], cand1[:, g, :],
                             start=(g == 0), stop=(g == NG - 1))
        nc.vector.tensor_copy(ob[:, C0:], sc1)
        nc.sync.dma_start(out=out_d[:, C0:], in_=ob[:, C0:])


def _host_prep(sequence_output, attention, W_head, W_tail, prototypes,
               mention_pos):
    """Build the per-core input maps (numpy only)."""
    import ml_dtypes

    f8 = ml_dtypes.float8_e4m3
    seq = np.asarray(sequence_output, dtype=np.float32)
    att = np.asarray(attention, dtype=np.float32)
    wh = np.asarray(W_head, dtype=np.float32)
    wt = np.asarray(W_tail, dtype=np.float32)
    pro = np.asarray(prototypes, dtype=np.float32)
    pos = np.asarray(mention_pos)

    def tile_rows(m, dt=np.float16):  # [T*128, N] -> [128, T, N]
        t = m.shape[0] // 128
        r = m.reshape(t, 128, -1).transpose(1, 0, 2)
        if dt is f8:
            r = np.clip(r, -240.0, 240.0)
        return np.ascontiguousarray(r, dtype=dt)

    def w_tiles(w):  # ctx rows, *64, DoubleRow pairs: [128, 3, 2, H] fp8
        m = np.clip(w[H:] * np.float32(64.0), -240, 240)
        return np.ascontiguousarray(
            m.reshape(HT // 2, 2, 128, H).transpose(2, 0, 1, 3), dtype=f8)

    whc = w_tiles(wh)
    wtc = w_tiles(wt)

    in_maps = []
    for c in range(NCORES):
        b, q = divmod(c, Q)
        p_bq = pos[b, q]                       # [E, M]
        # attention gather + mention-sum: [NH, E, L] (scale dropped)
        g = att[b, q][:, p_bq, :]              # [NH, E, M, L]
        asum = g[:, :, 0, :] + g[:, :, 1, :]   # [NH, E, L]
        # normalized pair weights muln[l, e, f] = 1024 * G / S
        A = np.ascontiguousarray(asum.transpose(2, 1, 0))  # [L, E, NH]
        G = A @ A.transpose(0, 2, 1)                       # [L, E, E]
        S = G.sum(axis=0)                                  # [E, E]
        Gn = G * (np.float32(1024.0) / S)[None]
        # entity means and tanh-bias rows ep_w = 1024 * ent @ W_w[:H]
        ment = seq[b, q][p_bq]                 # [E, M, H]
        ent = (ment[:, 0, :] + ment[:, 1, :]) * np.float32(0.5)
        ep = np.stack([ent @ wh[:H], ent @ wt[:H]]) * np.float32(1024.0)
        # ep layout [128, NG, E]: ep_l[p, w*HT+ht2, e] = ep[w, e, ht2*128+p]
        ep_l = np.ascontiguousarray(
            ep.reshape(2, E, HT, 128).transpose(3, 0, 2, 1).reshape(
                128, NG, E), dtype=np.float16)
        ptT = tile_rows(pro[b].reshape(RP, 2 * H).T)       # [128, NG, RP]
        sm0 = np.concatenate(
            [seq[b, q], Gn[:, :EC, :].reshape(L, C0)], axis=1)  # [L, H+C0]
        in_maps.append({
            "sm0": tile_rows(sm0, f8),
            "mu1": tile_rows(Gn[:, EC:, EC:].reshape(L, C1), f8),
            "whc": whc,
            "wtc": wtc,
            "ptT": ptT,
            "ep": ep_l,
        })
    return in_maps


def kernel(sequence_output, attention, W_head, W_tail, prototypes,
           mention_pos):
    from concourse.bass_utils import run_bass_kernel_spmd

    if "nc" not in _CACHE:
        _CACHE["nc"] = _build_program()
    nc = _CACHE["nc"]

    in_maps = _host_prep(sequence_output, attention, W_head, W_tail,
                         prototypes, mention_pos)
    res = run_bass_kernel_spmd(nc, in_maps, core_ids=list(range(NCORES)))

    out = np.empty((B, Q, E, E, R), dtype=np.float32)
    for c in range(NCORES):
        b, q = divmod(c, Q)
        sc = res.results[c]["out"]             # [RP, EF]
        v = sc.reshape(R, P, 2, EC, E).max(axis=1)   # [R, 2, EC, E]
        out[b, q] = v.reshape(R, E, E).transpose(1, 2, 0)
    return out
